# revision 3
# baseline (speedup 1.0000x reference)
"""Criss-cross (axial) sparse-attention module, data-parallel over batch on 8 NeuronCores.

Contract: kernel(**inputs) takes FULL unsharded inputs (numpy), returns FULL output.
Sharding: B=8 images, one per core (batch data-parallel); all params replicated.
"""
import math
from functools import partial

import numpy as np
import jax
import jax.numpy as jnp

BN_EPS = 1e-5
LN_EPS = 1e-5

B, C, H, W = 8, 256, 128, 128
N_CORES = 8


def _sincos_pos_embed(h, w, d):
    dim = d // 2
    div = np.exp(np.arange(0, dim, 2, dtype=np.float32) * (-math.log(10000.0) / dim))
    ph = np.arange(h, dtype=np.float32)[:, None, None]
    pw = np.arange(w, dtype=np.float32)[None, :, None]
    pe = np.zeros((h, w, d), dtype=np.float32)
    pe[:, :, 0:dim:2] = np.broadcast_to(np.sin(ph * div), (h, w, div.shape[0]))
    pe[:, :, 1:dim:2] = np.broadcast_to(np.cos(ph * div), (h, w, div.shape[0]))
    pe[:, :, dim::2] = np.broadcast_to(np.sin(pw * div), (h, w, div.shape[0]))
    pe[:, :, dim + 1::2] = np.broadcast_to(np.cos(pw * div), (h, w, div.shape[0]))
    return np.transpose(pe, (2, 0, 1))  # (d, h, w)


_POS = _sincos_pos_embed(H, W, C)
_DIAG = np.where(np.eye(H, dtype=bool), np.float32(-1e30), np.float32(0.0))


def _per_image(x, qw, qb, kw, kb, vw, vb, se_w1, se_w2, gamma, pos, diag):
    # x: (C, H, W) one image on one core. Weights pre-folded with BN scale.
    x = x + pos
    # SE block
    y = jnp.mean(x, axis=(1, 2))                      # (C,)
    y = jax.nn.relu(se_w1 @ y)                        # (Cse,)
    y = jax.nn.sigmoid(se_w2 @ y)                     # (C,)
    x = x * y[:, None, None]

    xf = x.reshape(C, H * W)
    q = jax.nn.relu(qw @ xf + qb[:, None]).reshape(-1, H, W)   # (C8,H,W)
    k = jax.nn.relu(kw @ xf + kb[:, None]).reshape(-1, H, W)   # (C8,H,W)
    v = (vw @ xf + vb[:, None]).reshape(C, H, W)               # (C,H,W)

    # Criss-cross energies
    e_h = jnp.einsum('chw,cHw->hwH', q, k)            # (H,W,H')
    e_h = e_h + diag[:, None, :]
    e_w = jnp.einsum('chw,chW->hwW', q, k)            # (H,W,W')
    # joint softmax over concat axis without materializing concat
    m = jnp.maximum(e_h.max(axis=2), e_w.max(axis=2))  # (H,W)
    p_h = jnp.exp(e_h - m[:, :, None])
    p_w = jnp.exp(e_w - m[:, :, None])
    s = p_h.sum(axis=2) + p_w.sum(axis=2)             # (H,W)
    a_h = p_h / s[:, :, None]
    a_w = p_w / s[:, :, None]

    out_h = jnp.einsum('hwH,cHw->chw', a_h, v)
    out_w = jnp.einsum('hwW,chW->chw', a_w, v)
    z = x + gamma * (out_h + out_w)

    mu = jnp.mean(z)
    var = jnp.mean(jnp.square(z - mu))
    return (z - mu) / jnp.sqrt(var + LN_EPS)


@partial(jax.pmap, axis_name='b',
         in_axes=(0,) + (None,) * 11, out_axes=0)
def _pmapped(x, qw, qb, kw, kb, vw, vb, se_w1, se_w2, gamma, pos, diag):
    return _per_image(x[0], qw, qb, kw, kb, vw, vb, se_w1, se_w2, gamma, pos, diag)[None]


def kernel(x, q_w, q_b, qbn_g, qbn_b, k_w, k_b, kbn_g, kbn_b,
           v_w, v_b, vbn_g, vbn_b, se_w1, se_w2, gamma):
    # Fold eval-mode BatchNorm (running stats 0/1) into conv weight+bias:
    # y = (w@x + b) * g/sqrt(1+eps) + beta
    s = 1.0 / math.sqrt(1.0 + BN_EPS)
    qs = (qbn_g * s).astype(np.float32)
    ks = (kbn_g * s).astype(np.float32)
    vs = (vbn_g * s).astype(np.float32)
    qw = np.asarray(q_w) * qs[:, None]
    qb = np.asarray(q_b) * qs + np.asarray(qbn_b)
    kw = np.asarray(k_w) * ks[:, None]
    kb = np.asarray(k_b) * ks + np.asarray(kbn_b)
    vw = np.asarray(v_w) * vs[:, None]
    vb = np.asarray(v_b) * vs + np.asarray(vbn_b)

    xs = np.asarray(x, np.float32).reshape(B, 1, C, H, W)
    out = _pmapped(xs, qw, qb, kw, kb, vw, vb,
                   np.asarray(se_w1), np.asarray(se_w2),
                   np.float32(np.asarray(gamma)[0]), _POS, _DIAG)
    return np.asarray(out).reshape(B, C, H, W).astype(np.float32)
